# revision 18
# baseline (speedup 1.0000x reference)
"""Trainium2 Bass kernel for nn_BranchRoute (threshold MoE routing).

reference:
    score = sigmoid(x @ W_gate + b_gate)          # [N, 2]
    hot   = score > 0.5                           # == (x @ W_gate + b_gate) > 0
    x_0   = where(hot[:, 0:1], x, 0)
    x_1   = where(hot[:, 1:2], x, 0)
    x_comb = x_0 + x_1

Sharding: data-parallel over tokens across 8 NeuronCores (2048 tokens/core),
gate weights replicated.

The kernel is DMA-bound, so HBM bytes are minimized within the harness's
rel-err<2e-2 budget: the three outputs are written as fp16 (rel err ~2e-4,
upcast to f32 on the host), and optionally x itself is shipped as fp16
(X_DTYPE="f16"), with the gate computed from fp16 x against f32 W.  The
fp16 gate flips 1 borderline token per branch for this problem's fixed
inputs (~1.1e-2 rel err) -- verified against the reference before enabling.

Per core the kernel streams 8 pair-tiles of [128 tokens, 2, 1024 d]:
 - gate logits z[:, br] by fused multiply+reduce on DVE (stt is DVE-only
   on core v3),
 - masks m0/m1 (z > -b) and mc = m0+m1 as tiny per-pair DVE ops,
 - o0/o1 on ACT (per-partition scalar mul, fp16 out), oc on DVE
   (tensor_scalar_mul runs at 2x for f32 SBUF operands),
 - loads prefetch on the Pool SWDGE queue (first pair on the SP HWDGE to
   cut the startup stall), o0/o1 stores on the SP HWDGE queue, oc stores
   on the ACT HWDGE queue.
"""

import numpy as np

N_TOKENS = 16384
D_MODEL = 1024
N_BRANCHES = 2
N_CORES = 8
N_SHARD = N_TOKENS // N_CORES  # 2048 tokens per core
P = 128                        # SBUF partitions
NTILES = N_SHARD // P          # 16 token-tiles per core

import os
X_DTYPE = os.environ.get("BR_XDT", "f32")  # "f32" (safe) or "f16"
GATE = os.environ.get("BR_GATE", "dve")    # "dve" or "pe"

_CACHE = {}


def _split_multi_waits(nc, max_embedded=1):
    """This container's walrus build rejects instructions carrying more than
    one embedded semaphore wait ("Too many sync wait commands").  Hoist the
    extra waits into standalone EventSemaphore instructions immediately
    before the owning instruction on the same engine -- identical ordering
    semantics, encodable by this compiler."""
    from concourse import mybir

    wid = 0
    for fn in nc.m.functions:
        for bb in fn.blocks:
            out = []
            changed = False
            for inst in bb.instructions:
                si = getattr(inst, "sync_info", None)
                waits = list(si.on_wait) if si is not None else []
                if si is not None and len(waits) > max_embedded:
                    extra, keep = waits[:-max_embedded], waits[-max_embedded:]
                    for w in extra:
                        es = mybir.InstEventSemaphore(
                            name=f"WSPLIT-{wid}", ins=[], outs=[]
                        )
                        wid += 1
                        es.engine = inst.engine
                        es.sync_info = mybir.SyncInfo(on_wait=[w], on_update=[])
                        out.append(es)
                    si.on_wait = keep
                    changed = True
                out.append(inst)
            if changed:
                bb.instructions = out


def _build_bass(x_dtype="f32"):
    import concourse.bass as bass
    import concourse.tile as tile
    from concourse import mybir

    f32 = mybir.dt.float32
    f16 = mybir.dt.float16
    xdt = f32 if x_dtype == "f32" else f16
    nc = bass.Bass(trn_type="TRN2", num_swdge_queues=2)

    # w is passed host-side as [N_BRANCHES, D_MODEL + 1]: row br holds
    # W[:, br] transposed (contiguous, so the partition-broadcast DMA reads
    # large bursts) with -b[br] appended as the last column.
    DW = D_MODEL + 1
    x_h = nc.dram_tensor("x", [N_SHARD, D_MODEL], xdt, kind="ExternalInput")
    w_h = nc.dram_tensor("w", [N_BRANCHES, DW], xdt, kind="ExternalInput")
    o0_h = nc.dram_tensor("o0", [N_SHARD, D_MODEL], f16, kind="ExternalOutput")
    o1_h = nc.dram_tensor("o1", [N_SHARD, D_MODEL], f16, kind="ExternalOutput")
    oc_h = nc.dram_tensor("oc", [N_SHARD, D_MODEL], f16, kind="ExternalOutput")

    # Pair token-tiles: [npair, 128, 2, 1024] -- one DMA per pair, partition
    # dim leading on both sides so the DMA APs balance.
    TB = 2
    NPAIR = NTILES // TB
    x_t = x_h[:].rearrange("(t s p) d -> t p s d", s=TB, p=P)
    o0_t = o0_h[:].rearrange("(t s p) d -> t p s d", s=TB, p=P)
    o1_t = o1_h[:].rearrange("(t s p) d -> t p s d", s=TB, p=P)
    oc_t = oc_h[:].rearrange("(t s p) d -> t p s d", s=TB, p=P)

    with tile.TileContext(nc) as tc:
        with (
            tc.tile_pool(name="singles", bufs=1) as singles,
            tc.tile_pool(name="xp", bufs=6) as xp,
            tc.tile_pool(name="scrd", bufs=3) as scrd,
            tc.tile_pool(name="out0", bufs=6) as p0,
            tc.tile_pool(name="out1", bufs=6) as p1,
            tc.tile_pool(name="outc", bufs=6) as pc,
            tc.tile_pool(name="small", bufs=8) as small,
        ):
            # [W^T | -b] rows broadcast across all 128 partitions.  A single
            # 0-partition-step DRAM broadcast DMA stalls startup, so split it
            # into 4 concurrent 32-partition chunks on the ACT HWDGE queue
            # (the SP queue carries the first x load).
            wb = singles.tile([P, N_BRANCHES * DW], xdt)
            w_ap = w_h[:]
            PCHUNK = 32
            for ci in range(P // PCHUNK):
                w_bcast = bass.AP(
                    tensor=w_ap.tensor,
                    offset=w_ap.offset,
                    ap=[[0, PCHUNK], [1, N_BRANCHES * DW]],
                )
                nc.scalar.dma_start(
                    out=wb[ci * PCHUNK : (ci + 1) * PCHUNK, :], in_=w_bcast
                )
            # negbf[p, br] = -b[br], in f32 (one-time upcast copy from the
            # strided -b columns of wb)
            negb_view = bass.AP(
                tensor=wb.tensor,
                offset=wb.offset + D_MODEL,
                ap=[wb.ap[0], [DW, N_BRANCHES]],
            )
            negbf = singles.tile([P, N_BRANCHES], f32)
            nc.scalar.copy(out=negbf, in_=negb_view)

            for i in range(NPAIR):
                x_sb = xp.tile([P, TB, D_MODEL], xdt)
                ld = nc.sync if i == 0 else nc.gpsimd
                ld.dma_start(out=x_sb, in_=x_t[i])

                # z[p, 2s+br] = sum_d x[p, s, d] * W[d, br] -- fused DVE
                # multiply+reduce (stt is DVE-only on core v3)
                z = small.tile([P, 2 * TB], f32)
                for s in range(TB):
                    x_s = x_sb[:, s, :]
                    for br in range(N_BRANCHES):
                        scr = scrd.tile([P, D_MODEL], f32)
                        nc.vector.scalar_tensor_tensor(
                            out=scr,
                            in0=x_s,
                            scalar=0.0,
                            in1=wb[:, br * DW : br * DW + D_MODEL],
                            op0=mybir.AluOpType.bypass,
                            op1=mybir.AluOpType.mult,
                            accum_out=z[:, 2 * s + br : 2 * s + br + 1],
                        )

                # masks for the whole pair in two DVE ops:
                # m[p, 2s+br] = (z > -b[br]),  mc[p, s] = m0 + m1
                m = small.tile([P, 2 * TB], f32)
                negb4 = bass.AP(
                    tensor=negbf.tensor,
                    offset=negbf.offset,
                    ap=[negbf.ap[0], [0, TB], [1, N_BRANCHES]],
                )
                nc.vector.tensor_tensor(
                    out=m, in0=z, in1=negb4, op=mybir.AluOpType.is_gt
                )
                mc = small.tile([P, TB], f32)
                m_ev = bass.AP(
                    tensor=m.tensor, offset=m.offset, ap=[m.ap[0], [2, TB]]
                )
                m_od = bass.AP(
                    tensor=m.tensor, offset=m.offset + 1, ap=[m.ap[0], [2, TB]]
                )
                nc.vector.tensor_tensor(
                    out=mc, in0=m_ev, in1=m_od, op=mybir.AluOpType.add
                )

                # masked fp16 outputs: o0/o1 on ACT, oc on DVE
                o0p = p0.tile([P, TB, D_MODEL], f16)
                o1p = p1.tile([P, TB, D_MODEL], f16)
                ocp = pc.tile([P, TB, D_MODEL], f16)
                for s in range(TB):
                    x_s = x_sb[:, s, :]
                    nc.scalar.mul(
                        out=o0p[:, s, :], in_=x_s, mul=m[:, 2 * s : 2 * s + 1]
                    )
                    nc.scalar.mul(
                        out=o1p[:, s, :], in_=x_s, mul=m[:, 2 * s + 1 : 2 * s + 2]
                    )
                    nc.vector.tensor_scalar_mul(
                        out=ocp[:, s, :], in0=x_s, scalar1=mc[:, s : s + 1]
                    )

                # stores: all on the SP HWDGE queue (keeps ACT free for the
                # o0/o1 multiplies; SP is otherwise idle)
                nc.sync.dma_start(out=o0_t[i], in_=o0p)
                nc.sync.dma_start(out=o1_t[i], in_=o1p)
                nc.sync.dma_start(out=oc_t[i], in_=ocp)

    _split_multi_waits(nc)
    return nc


def _build_bass_pe():
    """PE-gate variant (fp16 x only): the gate matmul runs on the otherwise
    idle PE engine -- per 128-token subtile, 8 PE transposes (identity
    trick) move x into d-major PSUM, ACT evacuates to SBUF fp16, then 8
    tiny PE matmuls against the W chunks accumulate z[tok, br] in PSUM.
    DVE only does the masks + all three fp16 multiplies (4x mode), so
    every engine sits well under the 16 MiB/core DMA floor."""
    import concourse.bass as bass
    import concourse.tile as tile
    from concourse import mybir

    f32 = mybir.dt.float32
    f16 = mybir.dt.float16
    nc = bass.Bass(trn_type="TRN2", num_swdge_queues=2)

    NCHUNK = D_MODEL // P  # 8 d-chunks of 128
    # x/o* use a partition-major quad layout [q, p, s, d] (host packs /
    # unpacks): each DMA descriptor covers one partition's contiguous
    # QB*2KB block instead of a single 2KB token row -- measured 2KB
    # descriptors cap the 16 DMA engines at ~280 GB/s aggregate.
    QB = 4
    NQUAD = NTILES // QB
    x_h = nc.dram_tensor("x", [NQUAD, P, QB, D_MODEL], f16, kind="ExternalInput")
    # wt2[p, c, br] = W[c*128 + p, br]  (natural partition layout, no bcast)
    w_h = nc.dram_tensor("wt2", [P, NCHUNK * N_BRANCHES], f16, kind="ExternalInput")
    # nb[0, br] = -b[br]
    nb_h = nc.dram_tensor("nb", [1, N_BRANCHES], f32, kind="ExternalInput")
    id_h = nc.dram_tensor("ident", [P, P], f16, kind="ExternalInput")
    o0_h = nc.dram_tensor("o0", [NQUAD, P, QB, D_MODEL], f16, kind="ExternalOutput")
    o1_h = nc.dram_tensor("o1", [NQUAD, P, QB, D_MODEL], f16, kind="ExternalOutput")
    oc_h = nc.dram_tensor("oc", [NQUAD, P, QB, D_MODEL], f16, kind="ExternalOutput")

    x_t = x_h[:]
    o0_t = o0_h[:]
    o1_t = o1_h[:]
    oc_t = oc_h[:]

    with tile.TileContext(nc) as tc:
        with (
            tc.tile_pool(name="singles", bufs=1) as singles,
            tc.tile_pool(name="xp", bufs=3) as xp,
            tc.tile_pool(name="xtp", bufs=3) as xtp,
            tc.tile_pool(name="out0", bufs=3) as p0,
            tc.tile_pool(name="out1", bufs=3) as p1,
            tc.tile_pool(name="outc", bufs=3) as pc,
            tc.tile_pool(name="small", bufs=8) as small,
            tc.tile_pool(name="psT", bufs=2, space="PSUM") as psT,
            tc.tile_pool(name="psZ", bufs=2, space="PSUM") as psZ,
        ):
            # one-time small loads on the ACT queue (SP carries the first
            # x pair): W chunks, -b broadcast, identity
            w2 = singles.tile([P, NCHUNK, N_BRANCHES], f16)
            nc.scalar.dma_start(out=w2, in_=w_h[:])
            ident = singles.tile([P, P], f16)
            nc.scalar.dma_start(out=ident, in_=id_h[:])
            negb = singles.tile([P, N_BRANCHES], f32)
            nb_ap = nb_h[:]
            nb_bcast = bass.AP(
                tensor=nb_ap.tensor, offset=nb_ap.offset,
                ap=[[0, P], [1, N_BRANCHES]],
            )
            nc.scalar.dma_start(out=negb, in_=nb_bcast)

            for i in range(NQUAD):
                x_sb = xp.tile([P, QB, D_MODEL], f16)
                if i == 0:
                    # split the first quad into per-subtile loads on the SP
                    # queue so compute ramps before the full 1 MiB arrives
                    for s in range(QB):
                        nc.sync.dma_start(
                            out=x_sb[:, s, :], in_=x_t[0][:, s, :]
                        )
                else:
                    nc.gpsimd.dma_start(out=x_sb, in_=x_t[i])

                o0p = p0.tile([P, QB, D_MODEL], f16)
                o1p = p1.tile([P, QB, D_MODEL], f16)
                ocp = pc.tile([P, QB, D_MODEL], f16)

                for s in range(QB):
                    x_s = x_sb[:, s, :]

                    # xT[d % 128, c, tok] = x[tok, c*128 + d%128] via 8 PE
                    # transposes into one fp16 PSUM bank, evacuated by ACT
                    xt_ps = psT.tile([P, NCHUNK, P], f16)
                    for c in range(NCHUNK):
                        nc.tensor.transpose(
                            xt_ps[:, c, :], x_s[:, c * P : (c + 1) * P], ident
                        )
                    xt_sb = xtp.tile([P, NCHUNK, P], f16)
                    nc.scalar.copy(out=xt_sb, in_=xt_ps)

                    # z[tok, br] = sum_c xT_c.T @ W_c  accumulated in PSUM
                    z_ps = psZ.tile([P, N_BRANCHES], f32)
                    for c in range(NCHUNK):
                        nc.tensor.matmul(
                            z_ps,
                            lhsT=xt_sb[:, c, :],
                            rhs=w2[:, c, :],
                            start=(c == 0),
                            stop=(c == NCHUNK - 1),
                        )

                    # masks: m[p, br] = z > -b[br]; mc = m0 + m1
                    m = small.tile([P, N_BRANCHES], f32)
                    nc.vector.tensor_tensor(
                        out=m, in0=z_ps, in1=negb, op=mybir.AluOpType.is_gt
                    )
                    mc = small.tile([P, 1], f32)
                    nc.vector.tensor_tensor(
                        out=mc, in0=m[:, 0:1], in1=m[:, 1:2],
                        op=mybir.AluOpType.add,
                    )

                    # all three masked fp16 outputs on DVE (4x mode)
                    nc.vector.tensor_scalar_mul(
                        out=o0p[:, s, :], in0=x_s, scalar1=m[:, 0:1]
                    )
                    nc.vector.tensor_scalar_mul(
                        out=o1p[:, s, :], in0=x_s, scalar1=m[:, 1:2]
                    )
                    nc.vector.tensor_scalar_mul(
                        out=ocp[:, s, :], in0=x_s, scalar1=mc
                    )

                # stores spread over three DMA queues: o0 on SP, o1 on ACT,
                # oc alternating SP / Pool-SWDGE (a single queue backlogs --
                # measured ~210 GB/s per queue)
                nc.sync.dma_start(out=o0_t[i], in_=o0p)
                nc.scalar.dma_start(out=o1_t[i], in_=o1p)
                qc = nc.sync if i % 2 == 0 else nc.gpsimd
                qc.dma_start(out=oc_t[i], in_=ocp)

    _split_multi_waits(nc)
    return nc


def _get_nc():
    key = (X_DTYPE, GATE)
    if key not in _CACHE:
        _CACHE[key] = _build_bass_pe() if GATE == "pe" else _build_bass(X_DTYPE)
    return _CACHE[key]


LAST_EXEC_NS = None
LAST_TRACE = None


def _ensure_ntff_shim():
    """antenv.axon_hooks is absent in this container image; when tracing is
    active (trace=True or BASS_TRACE set) run_bass_kernel_spmd imports it.
    Recreate it from the ctypes implementation shipped in trn_agent_boot."""
    import sys
    import types

    try:
        from antenv.axon_hooks import get_axon_ntff_profile_hook  # noqa: F401

        return
    except ImportError:
        pass
    try:
        from trn_agent_boot.trn_boot import _ntff_profile_via_ctypes

        hook = _ntff_profile_via_ctypes("/opt/axon/libaxon_pjrt.so")
    except Exception:
        hook = None
    mod = types.ModuleType("antenv.axon_hooks")
    mod.get_axon_ntff_profile_hook = lambda: hook
    sys.modules["antenv.axon_hooks"] = mod


def kernel(x, W_gate, b_gate, _trace=False):
    global LAST_EXEC_NS, LAST_TRACE
    import os

    from concourse.bass_utils import run_bass_kernel_spmd

    if _trace or os.environ.get("BASS_TRACE"):
        _ensure_ntff_shim()

    nc = _get_nc()
    if GATE == "pe":
        QB = 4
        NQUAD = NTILES // QB
        # pack [core, quad, p, s, d] partition-major (8KB DMA descriptors)
        xq = np.ascontiguousarray(
            np.asarray(x, dtype=np.float32)
            .astype(np.float16)
            .reshape(N_CORES, NQUAD, QB, P, D_MODEL)
            .transpose(0, 1, 3, 2, 4)
        )
        W = np.asarray(W_gate, dtype=np.float32).astype(np.float16)  # [D, NB]
        wt2 = np.ascontiguousarray(
            W.reshape(D_MODEL // P, P, N_BRANCHES).transpose(1, 0, 2).reshape(
                P, -1
            )
        )
        nb = -np.asarray(b_gate, dtype=np.float32).reshape(1, N_BRANCHES)
        ident = np.eye(P, dtype=np.float16)
        common = {"wt2": wt2, "nb": nb, "ident": ident}
        in_maps = [{"x": xq[c], **common} for c in range(N_CORES)]
    else:
        np_xdt = np.float32 if X_DTYPE == "f32" else np.float16
        x = np.ascontiguousarray(np.asarray(x, dtype=np.float32)).astype(np_xdt)
        wt = np.asarray(W_gate, dtype=np.float32).T  # [NB, D]
        negb = -np.asarray(b_gate, dtype=np.float32).reshape(N_BRANCHES, 1)
        w = np.ascontiguousarray(np.concatenate([wt, negb], axis=1)).astype(
            np_xdt
        )
        in_maps = [
            {"x": x[c * N_SHARD : (c + 1) * N_SHARD], "w": w}
            for c in range(N_CORES)
        ]
    res = run_bass_kernel_spmd(
        nc, in_maps, core_ids=list(range(N_CORES)), trace=_trace
    )
    LAST_EXEC_NS = res.exec_time_ns
    LAST_TRACE = getattr(res, "instructions_and_trace", None)

    if GATE == "pe":
        QB = 4
        NQUAD = NTILES // QB

        def unpack(name):
            a = np.stack([res.results[c][name] for c in range(N_CORES)])
            # [core, quad, p, s, d] -> token = c*2048 + q*512 + s*128 + p
            return np.ascontiguousarray(
                a.reshape(N_CORES, NQUAD, P, QB, D_MODEL)
                .transpose(0, 1, 3, 2, 4)
                .reshape(N_TOKENS, D_MODEL)
            ).astype(np.float32)

        return (unpack("o0"), unpack("o1"), unpack("oc"))

    x0 = np.concatenate(
        [res.results[c]["o0"] for c in range(N_CORES)], axis=0
    ).astype(np.float32)
    x1 = np.concatenate(
        [res.results[c]["o1"] for c in range(N_CORES)], axis=0
    ).astype(np.float32)
    xc = np.concatenate(
        [res.results[c]["oc"] for c in range(N_CORES)], axis=0
    ).astype(np.float32)
    return (x0, x1, xc)


# revision 21
# speedup vs baseline: 1.0718x; 1.0718x over previous
"""Trainium2 Bass kernel for nn_BranchRoute (threshold MoE routing).

reference:
    score = sigmoid(x @ W_gate + b_gate)          # [N, 2]
    hot   = score > 0.5                           # == (x @ W_gate + b_gate) > 0
    x_0   = where(hot[:, 0:1], x, 0)
    x_1   = where(hot[:, 1:2], x, 0)
    x_comb = x_0 + x_1

Sharding: data-parallel over tokens across 8 NeuronCores (2048 tokens/core),
gate weights replicated.

The kernel is DMA-bound, so HBM bytes are minimized within the harness's
rel-err<2e-2 budget: the three outputs are written as fp16 (rel err ~2e-4,
upcast to f32 on the host), and optionally x itself is shipped as fp16
(X_DTYPE="f16"), with the gate computed from fp16 x against f32 W.  The
fp16 gate flips 1 borderline token per branch for this problem's fixed
inputs (~1.1e-2 rel err) -- verified against the reference before enabling.

Per core the kernel streams 8 pair-tiles of [128 tokens, 2, 1024 d]:
 - gate logits z[:, br] by fused multiply+reduce on DVE (stt is DVE-only
   on core v3),
 - masks m0/m1 (z > -b) and mc = m0+m1 as tiny per-pair DVE ops,
 - o0/o1 on ACT (per-partition scalar mul, fp16 out), oc on DVE
   (tensor_scalar_mul runs at 2x for f32 SBUF operands),
 - loads prefetch on the Pool SWDGE queue (first pair on the SP HWDGE to
   cut the startup stall), o0/o1 stores on the SP HWDGE queue, oc stores
   on the ACT HWDGE queue.
"""

import numpy as np

N_TOKENS = 16384
D_MODEL = 1024
N_BRANCHES = 2
N_CORES = 8
N_SHARD = N_TOKENS // N_CORES  # 2048 tokens per core
P = 128                        # SBUF partitions
NTILES = N_SHARD // P          # 16 token-tiles per core

import os
X_DTYPE = os.environ.get("BR_XDT", "f32")  # "f32" (safe) or "f16"
GATE = os.environ.get("BR_GATE", "dve")    # "dve" or "pe"

_CACHE = {}


def _split_multi_waits(nc, max_embedded=1):
    """This container's walrus build rejects instructions carrying more than
    one embedded semaphore wait ("Too many sync wait commands").  Hoist the
    extra waits into standalone EventSemaphore instructions immediately
    before the owning instruction on the same engine -- identical ordering
    semantics, encodable by this compiler."""
    from concourse import mybir

    wid = 0
    for fn in nc.m.functions:
        for bb in fn.blocks:
            out = []
            changed = False
            for inst in bb.instructions:
                si = getattr(inst, "sync_info", None)
                waits = list(si.on_wait) if si is not None else []
                if si is not None and len(waits) > max_embedded:
                    extra, keep = waits[:-max_embedded], waits[-max_embedded:]
                    for w in extra:
                        es = mybir.InstEventSemaphore(
                            name=f"WSPLIT-{wid}", ins=[], outs=[]
                        )
                        wid += 1
                        es.engine = inst.engine
                        es.sync_info = mybir.SyncInfo(on_wait=[w], on_update=[])
                        out.append(es)
                    si.on_wait = keep
                    changed = True
                out.append(inst)
            if changed:
                bb.instructions = out


def _build_bass(x_dtype="f32"):
    import concourse.bass as bass
    import concourse.tile as tile
    from concourse import mybir

    f32 = mybir.dt.float32
    f16 = mybir.dt.float16
    xdt = f32 if x_dtype == "f32" else f16
    nc = bass.Bass(trn_type="TRN2", num_swdge_queues=2)

    # w is passed host-side as [N_BRANCHES, D_MODEL + 1]: row br holds
    # W[:, br] transposed (contiguous, so the partition-broadcast DMA reads
    # large bursts) with -b[br] appended as the last column.
    DW = D_MODEL + 1
    x_h = nc.dram_tensor("x", [N_SHARD, D_MODEL], xdt, kind="ExternalInput")
    w_h = nc.dram_tensor("w", [N_BRANCHES, DW], xdt, kind="ExternalInput")
    o0_h = nc.dram_tensor("o0", [N_SHARD, D_MODEL], f16, kind="ExternalOutput")
    o1_h = nc.dram_tensor("o1", [N_SHARD, D_MODEL], f16, kind="ExternalOutput")
    oc_h = nc.dram_tensor("oc", [N_SHARD, D_MODEL], f16, kind="ExternalOutput")

    # Pair token-tiles: [npair, 128, 2, 1024] -- one DMA per pair, partition
    # dim leading on both sides so the DMA APs balance.
    TB = 2
    NPAIR = NTILES // TB
    x_t = x_h[:].rearrange("(t s p) d -> t p s d", s=TB, p=P)
    o0_t = o0_h[:].rearrange("(t s p) d -> t p s d", s=TB, p=P)
    o1_t = o1_h[:].rearrange("(t s p) d -> t p s d", s=TB, p=P)
    oc_t = oc_h[:].rearrange("(t s p) d -> t p s d", s=TB, p=P)

    with tile.TileContext(nc) as tc:
        with (
            tc.tile_pool(name="singles", bufs=1) as singles,
            tc.tile_pool(name="xp", bufs=6) as xp,
            tc.tile_pool(name="scrd", bufs=3) as scrd,
            tc.tile_pool(name="out0", bufs=6) as p0,
            tc.tile_pool(name="out1", bufs=6) as p1,
            tc.tile_pool(name="outc", bufs=6) as pc,
            tc.tile_pool(name="small", bufs=8) as small,
        ):
            # [W^T | -b] rows broadcast across all 128 partitions.  A single
            # 0-partition-step DRAM broadcast DMA stalls startup, so split it
            # into 4 concurrent 32-partition chunks on the ACT HWDGE queue
            # (the SP queue carries the first x load).
            wb = singles.tile([P, N_BRANCHES * DW], xdt)
            w_ap = w_h[:]
            PCHUNK = 32
            for ci in range(P // PCHUNK):
                w_bcast = bass.AP(
                    tensor=w_ap.tensor,
                    offset=w_ap.offset,
                    ap=[[0, PCHUNK], [1, N_BRANCHES * DW]],
                )
                nc.scalar.dma_start(
                    out=wb[ci * PCHUNK : (ci + 1) * PCHUNK, :], in_=w_bcast
                )
            # negbf[p, br] = -b[br], in f32 (one-time upcast copy from the
            # strided -b columns of wb)
            negb_view = bass.AP(
                tensor=wb.tensor,
                offset=wb.offset + D_MODEL,
                ap=[wb.ap[0], [DW, N_BRANCHES]],
            )
            negbf = singles.tile([P, N_BRANCHES], f32)
            nc.scalar.copy(out=negbf, in_=negb_view)

            for i in range(NPAIR):
                x_sb = xp.tile([P, TB, D_MODEL], xdt)
                ld = nc.sync if i == 0 else nc.gpsimd
                ld.dma_start(out=x_sb, in_=x_t[i])

                # z[p, 2s+br] = sum_d x[p, s, d] * W[d, br] -- fused DVE
                # multiply+reduce (stt is DVE-only on core v3)
                z = small.tile([P, 2 * TB], f32)
                for s in range(TB):
                    x_s = x_sb[:, s, :]
                    for br in range(N_BRANCHES):
                        scr = scrd.tile([P, D_MODEL], f32)
                        nc.vector.scalar_tensor_tensor(
                            out=scr,
                            in0=x_s,
                            scalar=0.0,
                            in1=wb[:, br * DW : br * DW + D_MODEL],
                            op0=mybir.AluOpType.bypass,
                            op1=mybir.AluOpType.mult,
                            accum_out=z[:, 2 * s + br : 2 * s + br + 1],
                        )

                # masks for the whole pair in two DVE ops:
                # m[p, 2s+br] = (z > -b[br]),  mc[p, s] = m0 + m1
                m = small.tile([P, 2 * TB], f32)
                negb4 = bass.AP(
                    tensor=negbf.tensor,
                    offset=negbf.offset,
                    ap=[negbf.ap[0], [0, TB], [1, N_BRANCHES]],
                )
                nc.vector.tensor_tensor(
                    out=m, in0=z, in1=negb4, op=mybir.AluOpType.is_gt
                )
                mc = small.tile([P, TB], f32)
                m_ev = bass.AP(
                    tensor=m.tensor, offset=m.offset, ap=[m.ap[0], [2, TB]]
                )
                m_od = bass.AP(
                    tensor=m.tensor, offset=m.offset + 1, ap=[m.ap[0], [2, TB]]
                )
                nc.vector.tensor_tensor(
                    out=mc, in0=m_ev, in1=m_od, op=mybir.AluOpType.add
                )

                # masked fp16 outputs: o0/o1 on ACT, oc on DVE
                o0p = p0.tile([P, TB, D_MODEL], f16)
                o1p = p1.tile([P, TB, D_MODEL], f16)
                ocp = pc.tile([P, TB, D_MODEL], f16)
                for s in range(TB):
                    x_s = x_sb[:, s, :]
                    nc.scalar.mul(
                        out=o0p[:, s, :], in_=x_s, mul=m[:, 2 * s : 2 * s + 1]
                    )
                    nc.scalar.mul(
                        out=o1p[:, s, :], in_=x_s, mul=m[:, 2 * s + 1 : 2 * s + 2]
                    )
                    nc.vector.tensor_scalar_mul(
                        out=ocp[:, s, :], in0=x_s, scalar1=mc[:, s : s + 1]
                    )

                # stores: all on the SP HWDGE queue (keeps ACT free for the
                # o0/o1 multiplies; SP is otherwise idle)
                nc.sync.dma_start(out=o0_t[i], in_=o0p)
                nc.sync.dma_start(out=o1_t[i], in_=o1p)
                nc.sync.dma_start(out=oc_t[i], in_=ocp)

    _split_multi_waits(nc)
    return nc


def _build_bass_pe():
    """PE-gate variant (fp16 x only): the gate matmul runs on the otherwise
    idle PE engine -- per 128-token subtile, 8 PE transposes (identity
    trick) move x into d-major PSUM, ACT evacuates to SBUF fp16, then 8
    tiny PE matmuls against the W chunks accumulate z[tok, br] in PSUM.
    DVE only does the masks + all three fp16 multiplies (4x mode), so
    every engine sits well under the 16 MiB/core DMA floor."""
    import concourse.bass as bass
    import concourse.tile as tile
    from concourse import mybir

    f32 = mybir.dt.float32
    f16 = mybir.dt.float16
    nc = bass.Bass(trn_type="TRN2", num_swdge_queues=2)

    NCHUNK = D_MODEL // P  # 8 d-chunks of 128
    # x/o* use a partition-major quad layout [q, p, s, d] (host packs /
    # unpacks): each DMA descriptor covers one partition's contiguous
    # QB*2KB block instead of a single 2KB token row -- measured 2KB
    # descriptors cap the 16 DMA engines at ~280 GB/s aggregate.
    QB = 4
    NQUAD = NTILES // QB
    PB = 2
    NPAIR = NTILES // PB
    x_h = nc.dram_tensor("x", [NQUAD, P, QB, D_MODEL], f16, kind="ExternalInput")
    # wt2[p, c, br] = W[c*128 + p, br]  (natural partition layout, no bcast)
    w_h = nc.dram_tensor("wt2", [P, NCHUNK * N_BRANCHES], f16, kind="ExternalInput")
    # nb[0, br] = -b[br]
    nb_h = nc.dram_tensor("nb", [1, N_BRANCHES], f32, kind="ExternalInput")
    id_h = nc.dram_tensor("ident", [P, P], f16, kind="ExternalInput")
    # stores at pair granularity (finer drain pipeline than the quad loads)
    o0_h = nc.dram_tensor("o0", [NPAIR, P, PB, D_MODEL], f16, kind="ExternalOutput")
    o1_h = nc.dram_tensor("o1", [NPAIR, P, PB, D_MODEL], f16, kind="ExternalOutput")
    oc_h = nc.dram_tensor("oc", [NPAIR, P, PB, D_MODEL], f16, kind="ExternalOutput")

    x_t = x_h[:]
    o0_t = o0_h[:]
    o1_t = o1_h[:]
    oc_t = oc_h[:]

    with tile.TileContext(nc) as tc:
        with (
            tc.tile_pool(name="singles", bufs=1) as singles,
            tc.tile_pool(name="xp", bufs=3) as xp,
            tc.tile_pool(name="xtp", bufs=3) as xtp,
            tc.tile_pool(name="out0", bufs=3) as p0,
            tc.tile_pool(name="out1", bufs=3) as p1,
            tc.tile_pool(name="outc", bufs=3) as pc,
            tc.tile_pool(name="small", bufs=8) as small,
            tc.tile_pool(name="psT", bufs=2, space="PSUM") as psT,
            tc.tile_pool(name="psZ", bufs=2, space="PSUM") as psZ,
        ):
            # one-time small loads on the ACT queue (SP carries the first
            # x pair): W chunks, -b broadcast, identity
            w2 = singles.tile([P, NCHUNK, N_BRANCHES], f16)
            nc.scalar.dma_start(out=w2, in_=w_h[:])
            ident = singles.tile([P, P], f16)
            nc.scalar.dma_start(out=ident, in_=id_h[:])
            negb = singles.tile([P, N_BRANCHES], f32)
            nb_ap = nb_h[:]
            nb_bcast = bass.AP(
                tensor=nb_ap.tensor, offset=nb_ap.offset,
                ap=[[0, P], [1, N_BRANCHES]],
            )
            nc.scalar.dma_start(out=negb, in_=nb_bcast)

            for i in range(NQUAD):
                x_sb = xp.tile([P, QB, D_MODEL], f16)
                if i == 0:
                    # split the first quad into per-subtile loads on the SP
                    # queue so compute ramps before the full 1 MiB arrives
                    for s in range(QB):
                        nc.sync.dma_start(
                            out=x_sb[:, s, :], in_=x_t[0][:, s, :]
                        )
                else:
                    nc.gpsimd.dma_start(out=x_sb, in_=x_t[i])

                o0p = p0.tile([P, QB, D_MODEL], f16)
                o1p = p1.tile([P, QB, D_MODEL], f16)
                ocp = pc.tile([P, QB, D_MODEL], f16)

                for s in range(QB):
                    x_s = x_sb[:, s, :]

                    # xT[d % 128, c, tok] = x[tok, c*128 + d%128] via 8 PE
                    # transposes into one fp16 PSUM bank, evacuated by ACT
                    xt_ps = psT.tile([P, NCHUNK, P], f16)
                    for c in range(NCHUNK):
                        nc.tensor.transpose(
                            xt_ps[:, c, :], x_s[:, c * P : (c + 1) * P], ident
                        )
                    xt_sb = xtp.tile([P, NCHUNK, P], f16)
                    nc.scalar.copy(out=xt_sb, in_=xt_ps)

                    # z[tok, br] = sum_c xT_c.T @ W_c  accumulated in PSUM
                    z_ps = psZ.tile([P, N_BRANCHES], f32)
                    for c in range(NCHUNK):
                        nc.tensor.matmul(
                            z_ps,
                            lhsT=xt_sb[:, c, :],
                            rhs=w2[:, c, :],
                            start=(c == 0),
                            stop=(c == NCHUNK - 1),
                        )

                    # masks: m[p, br] = z > -b[br]; mc = m0 + m1
                    m = small.tile([P, N_BRANCHES], f32)
                    nc.vector.tensor_tensor(
                        out=m, in0=z_ps, in1=negb, op=mybir.AluOpType.is_gt
                    )
                    mc = small.tile([P, 1], f32)
                    nc.vector.tensor_tensor(
                        out=mc, in0=m[:, 0:1], in1=m[:, 1:2],
                        op=mybir.AluOpType.add,
                    )

                    # all three masked fp16 outputs on DVE (4x mode)
                    nc.vector.tensor_scalar_mul(
                        out=o0p[:, s, :], in0=x_s, scalar1=m[:, 0:1]
                    )
                    nc.vector.tensor_scalar_mul(
                        out=o1p[:, s, :], in0=x_s, scalar1=m[:, 1:2]
                    )
                    nc.vector.tensor_scalar_mul(
                        out=ocp[:, s, :], in0=x_s, scalar1=mc
                    )

                # stores at pair granularity, spread over three DMA queues:
                # o0 on SP, o1 on ACT, oc alternating SP / Pool-SWDGE
                for k in range(QB // PB):
                    j = (QB // PB) * i + k
                    sl = slice(k * PB, (k + 1) * PB)
                    nc.sync.dma_start(out=o0_t[j], in_=o0p[:, sl, :])
                    nc.scalar.dma_start(out=o1_t[j], in_=o1p[:, sl, :])
                    qc = nc.sync if j % 2 == 0 else nc.gpsimd
                    qc.dma_start(out=oc_t[j], in_=ocp[:, sl, :])

    _split_multi_waits(nc)
    return nc


def _get_nc():
    key = (X_DTYPE, GATE)
    if key not in _CACHE:
        _CACHE[key] = _build_bass_pe() if GATE == "pe" else _build_bass(X_DTYPE)
    return _CACHE[key]


LAST_EXEC_NS = None
LAST_TRACE = None


def _ensure_ntff_shim():
    """antenv.axon_hooks is absent in this container image; when tracing is
    active (trace=True or BASS_TRACE set) run_bass_kernel_spmd imports it.
    Recreate it from the ctypes implementation shipped in trn_agent_boot."""
    import sys
    import types

    try:
        from antenv.axon_hooks import get_axon_ntff_profile_hook  # noqa: F401

        return
    except ImportError:
        pass
    try:
        from trn_agent_boot.trn_boot import _ntff_profile_via_ctypes

        hook = _ntff_profile_via_ctypes("/opt/axon/libaxon_pjrt.so")
    except Exception:
        hook = None
    mod = types.ModuleType("antenv.axon_hooks")
    mod.get_axon_ntff_profile_hook = lambda: hook
    sys.modules["antenv.axon_hooks"] = mod


def kernel(x, W_gate, b_gate, _trace=False):
    global LAST_EXEC_NS, LAST_TRACE
    import os

    from concourse.bass_utils import run_bass_kernel_spmd

    if _trace or os.environ.get("BASS_TRACE"):
        _ensure_ntff_shim()

    nc = _get_nc()
    if GATE == "pe":
        QB = 4
        NQUAD = NTILES // QB
        # pack [core, quad, p, s, d] partition-major (8KB DMA descriptors)
        xq = np.ascontiguousarray(
            np.asarray(x, dtype=np.float32)
            .astype(np.float16)
            .reshape(N_CORES, NQUAD, QB, P, D_MODEL)
            .transpose(0, 1, 3, 2, 4)
        )
        W = np.asarray(W_gate, dtype=np.float32).astype(np.float16)  # [D, NB]
        wt2 = np.ascontiguousarray(
            W.reshape(D_MODEL // P, P, N_BRANCHES).transpose(1, 0, 2).reshape(
                P, -1
            )
        )
        nb = -np.asarray(b_gate, dtype=np.float32).reshape(1, N_BRANCHES)
        ident = np.eye(P, dtype=np.float16)
        common = {"wt2": wt2, "nb": nb, "ident": ident}
        in_maps = [{"x": xq[c], **common} for c in range(N_CORES)]
    else:
        np_xdt = np.float32 if X_DTYPE == "f32" else np.float16
        x = np.ascontiguousarray(np.asarray(x, dtype=np.float32)).astype(np_xdt)
        wt = np.asarray(W_gate, dtype=np.float32).T  # [NB, D]
        negb = -np.asarray(b_gate, dtype=np.float32).reshape(N_BRANCHES, 1)
        w = np.ascontiguousarray(np.concatenate([wt, negb], axis=1)).astype(
            np_xdt
        )
        in_maps = [
            {"x": x[c * N_SHARD : (c + 1) * N_SHARD], "w": w}
            for c in range(N_CORES)
        ]
    res = run_bass_kernel_spmd(
        nc, in_maps, core_ids=list(range(N_CORES)), trace=_trace
    )
    LAST_EXEC_NS = res.exec_time_ns
    LAST_TRACE = getattr(res, "instructions_and_trace", None)

    if GATE == "pe":
        PB = 2
        NPAIR = NTILES // PB

        def unpack(name):
            a = np.stack([res.results[c][name] for c in range(N_CORES)])
            # [core, pair, p, s, d] -> token = c*2048 + j*256 + s*128 + p
            return np.ascontiguousarray(
                a.reshape(N_CORES, NPAIR, P, PB, D_MODEL)
                .transpose(0, 1, 3, 2, 4)
                .reshape(N_TOKENS, D_MODEL)
            ).astype(np.float32)

        return (unpack("o0"), unpack("o1"), unpack("oc"))

    x0 = np.concatenate(
        [res.results[c]["o0"] for c in range(N_CORES)], axis=0
    ).astype(np.float32)
    x1 = np.concatenate(
        [res.results[c]["o1"] for c in range(N_CORES)], axis=0
    ).astype(np.float32)
    xc = np.concatenate(
        [res.results[c]["oc"] for c in range(N_CORES)], axis=0
    ).astype(np.float32)
    return (x0, x1, xc)
